# revision 21
# baseline (speedup 1.0000x reference)
"""Trainium2 Bass kernel: batched dot-product attention.

Problem: B=8, N=M=4096, D=64, fp32.
  out[b] = softmax(Q[b] @ K[b].T / sqrt(D)) @ V[b]

Sharding: batch b -> core b (8 cores, no communication).

Per-core algorithm (flash-attention, S^T layout, ACT-throughput bound):
  - Prologue (graduated pieces, fully overlapped with compute): load Q,K
    contiguously, cast bf16, bounce through DRAM staging [4096,128] and
    DMA-transpose into SBUF as Q^T/K^T [64,4096] (staging cols 64..127
    are never written; transposed garbage lands in unused partitions
    64..127 which no matmul reads). V: strided load into [128, chunk, 65]
    with ones in column 64 -> V'.
  - Main loop: OUTER over m-chunk groups (sizes 4,3,4,3,...,2,2 alternating
    between a 4-bank and a 3-bank PSUM pool), INNER over 8 query blocks
    (NB=512). K/V chunks for group g are only needed at ~g*15us, so the
    prologue never stalls compute.
      S^T[mchunk, nb] = (K^T_c).T @ Q^T_blk   (PE, bf16 -> PSUM fp32)
      P^T             = exp(scale * S^T)      (ACT, PSUM -> SBUF bf16)
      partial[nt,65]  = (P^T tile).T @ V'_c   (PE, per-group PSUM bank;
                                               col 64 = softmax denom)
      o_acc[j]       += partial               (DVE, SBUF f32 accumulator)
  - After the last group, per block: reciprocal of col 64,
    tensor_scalar_mul, one strided DMA store (streams with the last
    group's inner loop).
"""

import sys

import numpy as np

if "/opt/trn_rl_repo" not in sys.path:
    sys.path.insert(0, "/opt/trn_rl_repo")

import concourse.bass as bass
import concourse.tile as tile
from concourse import bacc, mybir
from concourse.tile import add_dep_helper
from concourse.bass_utils import run_bass_kernel_spmd
from concourse.masks import make_identity

B = 8
SEQ = 4096
D = 64
P = 128

F32 = mybir.dt.float32
BF16 = mybir.dt.bfloat16


def _group_sizes(n_mchunks):
    """Uniform groups of 3 m-chunks (last may be smaller): 32 -> [3]*10+[2].
    The S^T PSUM pools (two, alternating per inner iteration) are 3 banks
    each, leaving 2 banks for the PV partial pool: 3+3+2 = 8."""
    out, c = [], n_mchunks
    while c > 0:
        out.append(min(3, c))
        c -= out[-1]
    return out


def build_nc(seq=SEQ, nb=512):
    n_mchunks = seq // P
    n_blocks = seq // nb
    ntiles_blk = nb // P
    scale = 1.0 / np.sqrt(np.float32(D))
    gsizes = _group_sizes(n_mchunks)

    nc = bacc.Bacc("TRN2", target_bir_lowering=False, debug=False)

    q_dram = nc.dram_tensor("queries", [seq, D], F32, kind="ExternalInput")
    k_dram = nc.dram_tensor("keys", [seq, D], F32, kind="ExternalInput")
    v_dram = nc.dram_tensor("values", [seq, D], F32, kind="ExternalInput")
    o_dram = nc.dram_tensor("out", [seq, D], F32, kind="ExternalOutput")

    v_tiled = v_dram.ap().rearrange("(t p) d -> p t d", p=P)
    o_tiled = o_dram.ap().rearrange("(t p) d -> p t d", p=P)

    # graduated prologue pieces (rows); "pe" pieces skip the DRAM bounce
    # and use PE transposes (fast start), later pieces use DMA-transpose.
    if seq >= 4096:
        q_pe = [(0, 512), (512, 512), (1024, 1024)]
        q_dma = [(2048, seq - 2048)]
        k_pe = [(0, 512)]
        k_dma = [(512, 1024), (1536, seq - 1536)]
        v_pieces = [(0, 512), (512, 1024), (1536, seq - 1536)]
    else:
        q_pe = [(0, seq)]
        q_dma = []
        k_pe = [(0, seq)]
        k_dma = []
        v_pieces = [(0, seq)]

    with tile.TileContext(nc) as tc:
        with (
            tc.tile_pool(name="persist", bufs=1) as persist,
            tc.tile_pool(name="stage", bufs=2) as stage,
            tc.tile_pool(name="dstage", bufs=1, space="DRAM") as dstage,
            tc.tile_pool(name="pexp", bufs=3) as pexp,
            tc.tile_pool(name="outp", bufs=2) as outp,
            tc.tile_pool(name="small", bufs=2) as small,
            tc.tile_pool(name="sga", bufs=1, space=bass.MemorySpace.PSUM) as sgpa,
            tc.tile_pool(name="sgb", bufs=1, space=bass.MemorySpace.PSUM) as sgpb,
            tc.tile_pool(name="part", bufs=2, space=bass.MemorySpace.PSUM) as partp,
        ):
            qt_all = persist.tile([P, seq], BF16, tag="qt")
            kt_all = persist.tile([P, seq], BF16, tag="kt")
            v2 = persist.tile([P, n_mchunks, D + 1], BF16, tag="v2")
            oaccs = [
                persist.tile([P, ntiles_blk, D + 1], F32, tag=f"oa{j}", name=f"oa{j}")
                for j in range(n_blocks)
            ]
            qsd = dstage.tile([seq, P], BF16, tag="qsd")
            ksd = dstage.tile([seq, P], BF16, tag="ksd")
            id_bf = persist.tile([P, P], BF16, tag="idbf")
            make_identity(nc, id_bf)

            # ---------------- prologue (graduated pieces) ----------------
            def qk_piece(name, src, sd, dst, row0, nrows, after=None):
                rows = slice(row0, row0 + nrows)
                rpp = nrows // P
                src_t = src.ap()[rows, :].rearrange("(p r) d -> p r d", r=rpp)
                st_f = stage.tile([P, rpp, D], F32, tag=f"{name}sf", name=f"{name}sf", bufs=4 if name == "q" else 3)
                ld = nc.sync.dma_start(out=st_f, in_=src_t)
                if after is not None:
                    add_dep_helper(ld.ins, after.ins, sync=False,
                                   reason="prologue piece ordering")
                st_b = stage.tile([P, rpp, D], BF16, tag=f"{name}sb", name=f"{name}sb", bufs=4 if name == "q" else 3)
                nc.gpsimd.tensor_copy(st_b, st_f)
                sd_t = sd[rows, :].rearrange("(p r) d -> p r d", r=rpp)
                st = nc.sync.dma_start(out=sd_t[:, :, 0:D], in_=st_b)
                tr = nc.sync.dma_start_transpose(out=dst[:, rows], in_=sd[rows, :])
                return st, tr

            def v_piece(row0, nrows, after=None):
                vch = slice(row0 // P, (row0 + nrows) // P)
                npc = nrows // P
                v_f = stage.tile([P, npc, D + 1], F32, tag="vf", name="vf")
                ld = nc.sync.dma_start(
                    out=v_f[:, 0:npc, :][:, :, 0:D], in_=v_tiled[:, vch, :]
                )
                if after is not None:
                    add_dep_helper(ld.ins, after.ins, sync=False,
                                   reason="prologue piece ordering")
                nc.vector.memset(v_f[:, 0:npc, D : D + 1], 1.0)
                nc.gpsimd.tensor_copy(v2[:, vch, :], v_f[:, 0:npc, :])

            def qk_piece_pe(name, src, dst, row0, nrows):
                npc = nrows // P
                src_t = src.ap()[row0 : row0 + nrows, :].rearrange(
                    "(t p) d -> p t d", p=P
                )
                st_f = stage.tile([P, npc, D], F32, tag=f"{name}pf", name=f"{name}pf")
                nc.sync.dma_start(out=st_f, in_=src_t)
                st_b = stage.tile([P, npc, D], BF16, tag=f"{name}pb", name=f"{name}pb")
                nc.gpsimd.tensor_copy(st_b, st_f)
                for t in range(npc):
                    tp = partp.tile([D, P], BF16, tag="part", name="tp")
                    nc.tensor.transpose(tp, st_b[:, t, :], id_bf)
                    nc.vector.tensor_copy(
                        dst[0:D, row0 + t * P : row0 + (t + 1) * P], tp
                    )

            qk_piece_pe("q", q_dram, qt_all, *q_pe[0])
            qk_piece_pe("k", k_dram, kt_all, *k_pe[0])
            v_piece(*v_pieces[0])
            for pc in q_pe[1:2]:
                qk_piece_pe("q", q_dram, qt_all, *pc)
            gate = None
            for pc in q_dma:
                gate, _ = qk_piece("q", q_dram, qsd, qt_all, *pc)
            for pc in q_pe[2:]:
                qk_piece_pe("q", q_dram, qt_all, *pc)
            for i, pc in enumerate(k_dma):
                _, ktr = qk_piece("k", k_dram, ksd, kt_all, *pc, after=gate)
                v_piece(*v_pieces[1 + i], after=gate)
                gate = ktr

            # ---------------- main loop: outer m-groups, inner n-blocks ---
            n_groups = len(gsizes)
            mc = 0
            for gi, g in enumerate(gsizes):
                last_group = gi == n_groups - 1
                for j in range(n_blocks):
                    ncol = slice(j * nb, (j + 1) * nb)
                    it = gi * n_blocks + j
                    pool = sgpa if it % 2 == 0 else sgpb
                    s_g = pool.tile(
                        [P, g, nb], F32, tag=f"sg{it % 2}", name=f"sg{it % 2}"
                    )
                    for ci in range(g):
                        cc = mc + ci
                        nc.tensor.matmul(
                            s_g[:, ci, :],
                            kt_all[0:D, cc * P : (cc + 1) * P],
                            qt_all[0:D, ncol],
                            start=True,
                            stop=True,
                        )
                    p_g = pexp.tile([P, g, nb], BF16, tag="pg")
                    nc.scalar.activation(
                        out=p_g,
                        in_=s_g,
                        func=mybir.ActivationFunctionType.Exp,
                        scale=float(scale),
                    )
                    part = partp.tile([P, ntiles_blk, P], F32, tag="part")
                    for ci in range(g):
                        cc = mc + ci
                        for t in range(ntiles_blk):
                            nc.tensor.matmul(
                                part[:, t, 0 : D + 1],
                                p_g[:, ci, t * P : (t + 1) * P],
                                v2[:, cc, :],
                                start=(ci == 0 and t == 0),
                                stop=(ci == g - 1 and t == ntiles_blk - 1),
                                skip_group_check=True,
                            )
                    if gi == 0:
                        nc.vector.tensor_copy(oaccs[j], part[:, :, 0 : D + 1])
                    else:
                        nc.vector.tensor_add(
                            oaccs[j], oaccs[j], part[:, :, 0 : D + 1]
                        )

                    if last_group:
                        # epilogue for block j, streams with the inner loop
                        rinv = small.tile([P, ntiles_blk, 1], F32, tag="rinv")
                        nc.vector.reciprocal(rinv, oaccs[j][:, :, D : D + 1])
                        o_sb = outp.tile([P, ntiles_blk, D], F32, tag="osb")
                        rinv_b = bass.AP(
                            tensor=rinv.tensor,
                            offset=rinv.offset,
                            ap=[rinv.ap[0], rinv.ap[1], [0, D]],
                        )
                        nc.vector.tensor_tensor(
                            out=o_sb,
                            in0=oaccs[j][:, :, 0:D],
                            in1=rinv_b,
                            op=mybir.AluOpType.mult,
                        )
                        # alternate HWDGE queues so stores pipeline
                        eng = nc.sync if j % 2 == 0 else nc.scalar
                        eng.dma_start(
                            out=o_tiled[:, j * ntiles_blk : (j + 1) * ntiles_blk, :],
                            in_=o_sb,
                        )
                mc += g

    nc.compile()
    return nc


_NC_CACHE = {}


def _get_nc(**kw):
    key = tuple(sorted(kw.items()))
    if key not in _NC_CACHE:
        _NC_CACHE[key] = build_nc(**kw)
    return _NC_CACHE[key]


def kernel(queries, keys, values, **run_kwargs):
    """Full-input entry point: [8, 4096, 64] fp32 each -> [8, 4096, 64] fp32."""
    nc = _get_nc()
    in_maps = [
        {
            "queries": np.ascontiguousarray(queries[b], dtype=np.float32),
            "keys": np.ascontiguousarray(keys[b], dtype=np.float32),
            "values": np.ascontiguousarray(values[b], dtype=np.float32),
        }
        for b in range(B)
    ]
    res = run_bass_kernel_spmd(nc, in_maps, core_ids=list(range(B)), **run_kwargs)
    out = np.stack([res.results[b]["out"] for b in range(B)]).astype(np.float32)
    if run_kwargs:
        kernel.last_results = res
    return out


# revision 37
# speedup vs baseline: 9.2562x; 9.2562x over previous
"""Trainium2 Bass kernel: batched dot-product attention.

Problem: B=8, N=M=4096, D=64, fp32.
  out[b] = softmax(Q[b] @ K[b].T / sqrt(D)) @ V[b]

Sharding: batch b -> core b (8 cores, no communication).

Per-core algorithm (flash-attention, S^T layout, ACT/exp-throughput bound
-- 16.7M exps on the scalar engine ~ 110 us is the hard floor):
  - Prologue (graduated pieces, overlapped with compute): early Q/K pieces
    are transposed on the PE (fp32 transpose, bf16 cast in the PSUM->SBUF
    copy); the large tail pieces bounce through a bf16 DRAM staging
    [rows,128] and one hardware DMA-transpose each into SBUF Q^T/K^T
    [64,4096] (staging cols 64..127 never written; the transposed garbage
    lands in SBUF partitions 64..127 which nothing reads). V: strided
    load into [128, chunk, 65] with ones in column 64 -> V'.
  - Main loop: OUTER over m-chunk groups (3 chunks of 128 keys per exp
    instruction), INNER over 8 query blocks (NB=512). Two 3-bank S^T PSUM
    pools alternate by inner-iteration parity; with the 2-bank PV-partial
    pool that is exactly 8 PSUM banks. K/V chunks of group g are only
    needed at ~g*15us, so the prologue never stalls compute.
      S^T[mchunk, nb] = (K^T_c).T @ Q^T_blk   (PE, bf16 -> PSUM fp32)
      P^T             = exp(scale * S^T)      (ACT, PSUM -> SBUF bf16)
      partial[nt,65]  = (P^T tile).T @ V'_c   (PE, 1 PSUM bank;
                                               col 64 = softmax denom)
      o_acc[j]       += partial               (DVE, SBUF f32 accumulator)
  - Last group, per block (streams with the inner loop): reciprocal of
    the sums column, one broadcast tensor_tensor multiply (0-stride AP),
    one strided DMA store, alternating SP/ACT HWDGE queues.
"""

import sys

import numpy as np

if "/opt/trn_rl_repo" not in sys.path:
    sys.path.insert(0, "/opt/trn_rl_repo")

import concourse.bass as bass
import concourse.tile as tile
from concourse import bacc, mybir
from concourse.tile import add_dep_helper
from concourse.bass_utils import run_bass_kernel_spmd
from concourse.masks import make_identity

B = 8
SEQ = 4096
D = 64
P = 128

F32 = mybir.dt.float32
BF16 = mybir.dt.bfloat16


def _group_sizes(n_mchunks):
    """Uniform groups of 3 m-chunks (last may be smaller): 32 -> [3]*10+[2].
    The S^T PSUM pools (two, alternating per inner iteration) are 3 banks
    each, leaving 2 banks for the PV partial pool: 3+3+2 = 8."""
    out, c = [], n_mchunks
    while c > 0:
        out.append(min(3, c))
        c -= out[-1]
    return out


def build_nc(seq=SEQ, nb=512, iters=1):
    n_mchunks = seq // P
    n_blocks = seq // nb
    ntiles_blk = nb // P
    scale = 1.0 / np.sqrt(np.float32(D))
    gsizes = _group_sizes(n_mchunks)

    nc = bacc.Bacc("TRN2", target_bir_lowering=False, debug=False)

    q_dram = nc.dram_tensor("queries", [seq, D], F32, kind="ExternalInput")
    k_dram = nc.dram_tensor("keys", [seq, D], F32, kind="ExternalInput")
    v_dram = nc.dram_tensor("values", [seq, D], F32, kind="ExternalInput")
    o_dram = nc.dram_tensor("out", [seq, D], F32, kind="ExternalOutput")

    v_tiled = v_dram.ap().rearrange("(t p) d -> p t d", p=P)
    o_tiled = o_dram.ap().rearrange("(t p) d -> p t d", p=P)

    # graduated prologue pieces (rows). Q is entirely PE-transposed
    # (contiguous loads; late pieces are emitted woven between the first
    # main-loop iterations so the part-pool rotation never blocks PV).
    # K's tail goes through the DRAM staging + DMA-transpose path.
    if seq >= 4096:
        q_pe = [(0, 512), (512, 512), (1024, 1024)]
        q_weave = {0: (2048, 1024), 1: (3072, 1024)}
        k_pe = [(0, 512)]
        k_dma = [(512, 1024), (1536, seq - 1536)]
        v_pieces = [(0, 512), (512, 1024), (1536, seq - 1536)]
    else:
        q_pe = [(0, seq)]
        q_weave = {}
        k_pe = [(0, seq)]
        k_dma = []
        v_pieces = [(0, seq)]

    with tile.TileContext(nc) as tc:
        with (
            tc.tile_pool(name="persist", bufs=1) as persist,
            tc.tile_pool(name="stage", bufs=2) as stage,
            tc.tile_pool(name="dstage", bufs=1, space="DRAM") as dstage,
            tc.tile_pool(name="pexp", bufs=3) as pexp,
            tc.tile_pool(name="outp", bufs=4) as outp,
            tc.tile_pool(name="small", bufs=4) as small,
            tc.tile_pool(name="sga", bufs=1, space=bass.MemorySpace.PSUM) as sgpa,
            tc.tile_pool(name="sgb", bufs=1, space=bass.MemorySpace.PSUM) as sgpb,
            tc.tile_pool(name="part", bufs=2, space=bass.MemorySpace.PSUM) as partp,
        ):
            qt_all = persist.tile([P, seq], BF16, tag="qt")
            kt_all = persist.tile([P, seq], BF16, tag="kt")
            v2 = persist.tile([P, n_mchunks, D + 1], BF16, tag="v2")
            oaccs = [
                persist.tile([P, ntiles_blk, D + 1], F32, tag=f"oa{j}", name=f"oa{j}")
                for j in range(n_blocks)
            ]
            qsd = dstage.tile([seq, P], BF16, tag="qsd")
            ksd = dstage.tile([seq, P], BF16, tag="ksd")
            id_f32 = persist.tile([P, P], F32, tag="idf32")
            make_identity(nc, id_f32)

            # ---------------- prologue (graduated pieces) ----------------
            def qk_piece(name, src, sd, dst, row0, nrows, after=None):
                rows = slice(row0, row0 + nrows)
                rpp = nrows // P
                src_t = src.ap()[rows, :].rearrange("(p r) d -> p r d", r=rpp)
                st_f = stage.tile([P, rpp, D], F32, tag=f"{name}sf", name=f"{name}sf", bufs=4 if name == "q" else 3)
                ld = nc.sync.dma_start(out=st_f, in_=src_t)
                if after is not None:
                    add_dep_helper(ld.ins, after.ins, sync=False,
                                   reason="prologue piece ordering")
                st_b = stage.tile([P, rpp, D], BF16, tag=f"{name}sb", name=f"{name}sb", bufs=4 if name == "q" else 3)
                nc.gpsimd.tensor_copy(st_b, st_f)
                sd_t = sd[rows, :].rearrange("(p r) d -> p r d", r=rpp)
                st = nc.sync.dma_start(out=sd_t[:, :, 0:D], in_=st_b)
                tr = nc.sync.dma_start_transpose(out=dst[:, rows], in_=sd[rows, :])
                return st, tr

            def v_piece(row0, nrows, after=None):
                vch = slice(row0 // P, (row0 + nrows) // P)
                npc = nrows // P
                v_f = stage.tile([P, npc, D + 1], F32, tag="vf", name="vf")
                ld = nc.sync.dma_start(
                    out=v_f[:, 0:npc, :][:, :, 0:D], in_=v_tiled[:, vch, :]
                )
                if after is not None:
                    add_dep_helper(ld.ins, after.ins, sync=False,
                                   reason="prologue piece ordering")
                nc.vector.memset(v_f[:, 0:npc, D : D + 1], 1.0)
                nc.gpsimd.tensor_copy(v2[:, vch, :], v_f[:, 0:npc, :])

            def qk_piece_pe(name, src, dst, row0, nrows):
                # contiguous load: partition p <- rows [row0+p*rpp, ...).
                # The r-th slice [:, r, :] = rows {row0 + p*rpp + r} is one
                # PE-transpose unit whose output columns land strided (step
                # rpp) in dst.
                rpp = nrows // P
                src_t = src.ap()[row0 : row0 + nrows, :].rearrange(
                    "(p r) d -> p r d", r=rpp
                )
                st_f = stage.tile([P, rpp, D], F32, tag=f"{name}pf", name=f"{name}pf")
                ld = nc.sync.dma_start(out=st_f, in_=src_t)
                dst_v = dst[0:D, row0 : row0 + nrows].rearrange(
                    "d (p r) -> d r p", r=rpp
                )
                # quads of transposes share one PSUM slot; one copy per quad
                # (the copies are latency-dominated on DVE otherwise)
                for r0 in range(0, rpp, 4):
                    nq = min(4, rpp - r0)
                    tp = partp.tile([D, 4, P], F32, tag="part", name="tp")
                    for i in range(nq):
                        # fp32 PE transpose straight from the f32 staging;
                        # the PSUM->SBUF copy does the bf16 cast.
                        nc.tensor.transpose(tp[:, i, :], st_f[:, r0 + i, :], id_f32)
                    nc.vector.tensor_copy(
                        dst_v[:, r0 : r0 + nq, :], tp[:, 0:nq, :]
                    )
                return ld

            qk_piece_pe("q", q_dram, qt_all, *q_pe[0])
            qk_piece_pe("k", k_dram, kt_all, *k_pe[0])
            v_piece(*v_pieces[0])
            gate = None
            for pc in q_pe[1:]:
                gate = qk_piece_pe("q", q_dram, qt_all, *pc)
            for i, pc in enumerate(k_dma):
                _, ktr = qk_piece("k", k_dram, ksd, kt_all, *pc, after=gate)
                v_piece(*v_pieces[1 + i], after=gate)
                gate = ktr

            # ---------------- main loop: outer m-groups, inner n-blocks ---
            # (iters>1 repeats the whole main loop inside one NEFF for
            #  device-time measurement; results are idempotent)
            n_groups = len(gsizes)
            for _rep in range(iters):
              mc = 0
              for gi, g in enumerate(gsizes):
                  last_group = gi == n_groups - 1
                  for j in range(n_blocks):
                      ncol = slice(j * nb, (j + 1) * nb)
                      it = gi * n_blocks + j
                      if _rep == 0 and it in q_weave:
                          qk_piece_pe("q", q_dram, qt_all, *q_weave[it])
                      pool = sgpa if it % 2 == 0 else sgpb
                      s_g = pool.tile(
                          [P, g, nb], F32, tag=f"sg{it % 2}", name=f"sg{it % 2}"
                      )
                      for ci in range(g):
                          cc = mc + ci
                          nc.tensor.matmul(
                              s_g[:, ci, :],
                              kt_all[0:D, cc * P : (cc + 1) * P],
                              qt_all[0:D, ncol],
                              start=True,
                              stop=True,
                          )
                      p_g = pexp.tile([P, g, nb], BF16, tag="pg")
                      nc.scalar.activation(
                          out=p_g,
                          in_=s_g,
                          func=mybir.ActivationFunctionType.Exp,
                          scale=float(scale),
                      )
                      part = partp.tile([P, ntiles_blk, P], F32, tag="part")
                      for ci in range(g):
                          cc = mc + ci
                          for t in range(ntiles_blk):
                              nc.tensor.matmul(
                                  part[:, t, 0 : D + 1],
                                  p_g[:, ci, t * P : (t + 1) * P],
                                  v2[:, cc, :],
                                  start=(ci == 0 and t == 0),
                                  stop=(ci == g - 1 and t == ntiles_blk - 1),
                                  skip_group_check=True,
                              )
                      if gi == 0:
                          nc.vector.tensor_copy(oaccs[j], part[:, :, 0 : D + 1])
                      else:
                          nc.vector.tensor_add(
                              oaccs[j], oaccs[j], part[:, :, 0 : D + 1]
                          )

                      if last_group:
                          # epilogue for block j, streams with the inner loop
                          rinv = small.tile([P, ntiles_blk, 1], F32, tag="rinv")
                          nc.vector.reciprocal(rinv, oaccs[j][:, :, D : D + 1])
                          o_sb = outp.tile([P, ntiles_blk, D], F32, tag="osb")
                          rinv_b = bass.AP(
                              tensor=rinv.tensor,
                              offset=rinv.offset,
                              ap=[rinv.ap[0], rinv.ap[1], [0, D]],
                          )
                          nc.vector.tensor_tensor(
                              out=o_sb,
                              in0=oaccs[j][:, :, 0:D],
                              in1=rinv_b,
                              op=mybir.AluOpType.mult,
                          )
                          nc.sync.dma_start(
                              out=o_tiled[:, j * ntiles_blk : (j + 1) * ntiles_blk, :],
                              in_=o_sb,
                          )
                  mc += g

    nc.compile()
    return nc


_NC_CACHE = {}


def _get_nc(**kw):
    key = tuple(sorted(kw.items()))
    if key not in _NC_CACHE:
        _NC_CACHE[key] = build_nc(**kw)
    return _NC_CACHE[key]


def kernel(queries, keys, values, **run_kwargs):
    """Full-input entry point: [8, 4096, 64] fp32 each -> [8, 4096, 64] fp32."""
    nc = _get_nc()
    in_maps = [
        {
            "queries": np.ascontiguousarray(queries[b], dtype=np.float32),
            "keys": np.ascontiguousarray(keys[b], dtype=np.float32),
            "values": np.ascontiguousarray(values[b], dtype=np.float32),
        }
        for b in range(B)
    ]
    res = run_bass_kernel_spmd(nc, in_maps, core_ids=list(range(B)), **run_kwargs)
    out = np.stack([res.results[b]["out"] for b in range(B)]).astype(np.float32)
    if run_kwargs:
        kernel.last_results = res
    return out



# revision 42
# speedup vs baseline: 18.2210x; 1.9685x over previous
"""Trainium2 Bass kernel: batched dot-product attention.

Problem: B=8, N=M=4096, D=64, fp32.
  out[b] = softmax(Q[b] @ K[b].T / sqrt(D)) @ V[b]

Sharding: batch b -> core b (8 cores, no communication).

Per-core algorithm (flash-attention, S^T layout, ACT/exp-throughput bound
-- 16.7M exps on the scalar engine ~ 110 us is the hard floor):
  - Prologue (graduated pieces, overlapped with compute): early Q/K pieces
    are transposed on the PE (fp32 transpose, bf16 cast in the PSUM->SBUF
    copy); the large tail pieces bounce through a bf16 DRAM staging
    [rows,128] and one hardware DMA-transpose each into SBUF Q^T/K^T
    [64,4096] (staging cols 64..127 never written; the transposed garbage
    lands in SBUF partitions 64..127 which nothing reads). V: strided
    load into [128, chunk, 65] with ones in column 64 -> V'.
  - Main loop: OUTER over m-chunk groups (3 chunks of 128 keys per exp
    instruction), INNER over 8 query blocks (NB=512). Two 3-bank S^T PSUM
    pools alternate by inner-iteration parity; with the 2-bank PV-partial
    pool that is exactly 8 PSUM banks. K/V chunks of group g are only
    needed at ~g*15us, so the prologue never stalls compute.
      S^T[mchunk, nb] = (K^T_c).T @ Q^T_blk   (PE, bf16 -> PSUM fp32)
      P^T             = exp(scale * S^T)      (ACT, PSUM -> SBUF bf16)
      partial[nt,65]  = (P^T tile).T @ V'_c   (PE, 1 PSUM bank;
                                               col 64 = softmax denom)
      o_acc[j]       += partial               (DVE, SBUF f32 accumulator)
  - Last group, per block (streams with the inner loop): reciprocal of
    the sums column, one broadcast tensor_tensor multiply (0-stride AP),
    one strided DMA store on the SP HWDGE queue.
"""

import sys

import numpy as np

if "/opt/trn_rl_repo" not in sys.path:
    sys.path.insert(0, "/opt/trn_rl_repo")

import concourse.bass as bass
import concourse.tile as tile
from concourse import bacc, mybir
from concourse.tile import add_dep_helper
from concourse.bass_utils import run_bass_kernel_spmd
from concourse.masks import make_identity

B = 8
SEQ = 4096
D = 64
P = 128

F32 = mybir.dt.float32
BF16 = mybir.dt.bfloat16


def _group_sizes(n_mchunks):
    """Uniform groups of 3 m-chunks (last may be smaller): 32 -> [3]*10+[2].
    The S^T PSUM pools (two, alternating per inner iteration) are 3 banks
    each, leaving 2 banks for the PV partial pool: 3+3+2 = 8."""
    out, c = [], n_mchunks
    while c > 0:
        out.append(min(3, c))
        c -= out[-1]
    return out


def build_nc(seq=SEQ, nb=512, iters=1):
    n_mchunks = seq // P
    n_blocks = seq // nb
    ntiles_blk = nb // P
    scale = 1.0 / np.sqrt(np.float32(D))
    gsizes = _group_sizes(n_mchunks)

    nc = bacc.Bacc("TRN2", target_bir_lowering=False, debug=False)

    q_dram = nc.dram_tensor("queries", [seq, D], F32, kind="ExternalInput")
    k_dram = nc.dram_tensor("keys", [seq, D], F32, kind="ExternalInput")
    v_dram = nc.dram_tensor("values", [seq, D], F32, kind="ExternalInput")
    o_dram = nc.dram_tensor("out", [seq, D], F32, kind="ExternalOutput")

    v_tiled = v_dram.ap().rearrange("(t p) d -> p t d", p=P)
    o_tiled = o_dram.ap().rearrange("(t p) d -> p t d", p=P)

    # graduated prologue pieces (rows). Q is entirely PE-transposed
    # (contiguous loads; late pieces are emitted woven between the first
    # main-loop iterations so the part-pool rotation never blocks PV).
    # K's tail goes through the DRAM staging + DMA-transpose path.
    if seq >= 4096:
        q_pe = [(0, 512), (512, 512), (1024, 1024)]
        q_weave = {0: (2048, 1024), 1: (3072, 1024)}
        k_pe = [(0, 512)]
        k_dma = [(512, 1024), (1536, seq - 1536)]
        v_pieces = [(0, 512), (512, 1024), (1536, seq - 1536)]
    else:
        q_pe = [(0, seq)]
        q_weave = {}
        k_pe = [(0, seq)]
        k_dma = []
        v_pieces = [(0, seq)]

    with tile.TileContext(nc) as tc:
        with (
            tc.tile_pool(name="persist", bufs=1) as persist,
            tc.tile_pool(name="stage", bufs=2) as stage,
            tc.tile_pool(name="dstage", bufs=1, space="DRAM") as dstage,
            tc.tile_pool(name="pexp", bufs=3) as pexp,
            tc.tile_pool(name="outp", bufs=4) as outp,
            tc.tile_pool(name="small", bufs=4) as small,
            tc.tile_pool(name="sga", bufs=1, space=bass.MemorySpace.PSUM) as sgpa,
            tc.tile_pool(name="sgb", bufs=1, space=bass.MemorySpace.PSUM) as sgpb,
            tc.tile_pool(name="part", bufs=2, space=bass.MemorySpace.PSUM) as partp,
        ):
            qt_all = persist.tile([P, seq], BF16, tag="qt")
            kt_all = persist.tile([P, seq], BF16, tag="kt")
            v2 = persist.tile([P, n_mchunks, D + 1], BF16, tag="v2")
            oaccs = [
                persist.tile([P, ntiles_blk, D + 1], F32, tag=f"oa{j}", name=f"oa{j}")
                for j in range(n_blocks)
            ]
            qsd = dstage.tile([seq, P], BF16, tag="qsd")
            ksd = dstage.tile([seq, P], BF16, tag="ksd")
            id_f32 = persist.tile([P, P], F32, tag="idf32")
            make_identity(nc, id_f32)

            # ---------------- prologue (graduated pieces) ----------------
            def qk_piece(name, src, sd, dst, row0, nrows, after=None):
                rows = slice(row0, row0 + nrows)
                rpp = nrows // P
                src_t = src.ap()[rows, :].rearrange("(p r) d -> p r d", r=rpp)
                st_f = stage.tile([P, rpp, D], F32, tag=f"{name}sf", name=f"{name}sf", bufs=4 if name == "q" else 3)
                ld = nc.sync.dma_start(out=st_f, in_=src_t)
                if after is not None:
                    add_dep_helper(ld.ins, after.ins, sync=False,
                                   reason="prologue piece ordering")
                st_b = stage.tile([P, rpp, D], BF16, tag=f"{name}sb", name=f"{name}sb", bufs=4 if name == "q" else 3)
                nc.gpsimd.tensor_copy(st_b, st_f)
                sd_t = sd[rows, :].rearrange("(p r) d -> p r d", r=rpp)
                st = nc.sync.dma_start(out=sd_t[:, :, 0:D], in_=st_b)
                tr = nc.sync.dma_start_transpose(out=dst[:, rows], in_=sd[rows, :])
                return st, tr

            def v_piece(row0, nrows, after=None):
                vch = slice(row0 // P, (row0 + nrows) // P)
                npc = nrows // P
                v_f = stage.tile([P, npc, D + 1], F32, tag="vf", name="vf")
                ld = nc.sync.dma_start(
                    out=v_f[:, 0:npc, :][:, :, 0:D], in_=v_tiled[:, vch, :]
                )
                if after is not None:
                    add_dep_helper(ld.ins, after.ins, sync=False,
                                   reason="prologue piece ordering")
                nc.vector.memset(v_f[:, 0:npc, D : D + 1], 1.0)
                nc.gpsimd.tensor_copy(v2[:, vch, :], v_f[:, 0:npc, :])

            def qk_piece_pe(name, src, dst, row0, nrows):
                # contiguous load: partition p <- rows [row0+p*rpp, ...).
                # The r-th slice [:, r, :] = rows {row0 + p*rpp + r} is one
                # PE-transpose unit whose output columns land strided (step
                # rpp) in dst.
                rpp = nrows // P
                src_t = src.ap()[row0 : row0 + nrows, :].rearrange(
                    "(p r) d -> p r d", r=rpp
                )
                st_f = stage.tile([P, rpp, D], F32, tag=f"{name}pf", name=f"{name}pf")
                ld = nc.sync.dma_start(out=st_f, in_=src_t)
                dst_v = dst[0:D, row0 : row0 + nrows].rearrange(
                    "d (p r) -> d r p", r=rpp
                )
                # quads of transposes share one PSUM slot; one copy per quad
                # (the copies are latency-dominated on DVE otherwise)
                for r0 in range(0, rpp, 4):
                    nq = min(4, rpp - r0)
                    tp = partp.tile([D, 4, P], F32, tag="part", name="tp")
                    for i in range(nq):
                        # fp32 PE transpose straight from the f32 staging;
                        # the PSUM->SBUF copy does the bf16 cast.
                        nc.tensor.transpose(tp[:, i, :], st_f[:, r0 + i, :], id_f32)
                    nc.vector.tensor_copy(
                        dst_v[:, r0 : r0 + nq, :], tp[:, 0:nq, :]
                    )
                return ld

            qk_piece_pe("q", q_dram, qt_all, *q_pe[0])
            qk_piece_pe("k", k_dram, kt_all, *k_pe[0])
            v_piece(*v_pieces[0])
            gate = None
            for pc in q_pe[1:]:
                gate = qk_piece_pe("q", q_dram, qt_all, *pc)
            for i, pc in enumerate(k_dma):
                _, ktr = qk_piece("k", k_dram, ksd, kt_all, *pc, after=gate)
                v_piece(*v_pieces[1 + i], after=gate)
                gate = ktr

            # ---------------- main loop: outer m-groups, inner n-blocks ---
            # (iters>1 repeats the whole main loop inside one NEFF for
            #  device-time measurement; results are idempotent)
            n_groups = len(gsizes)
            for _rep in range(iters):
              mc = 0
              for gi, g in enumerate(gsizes):
                  last_group = gi == n_groups - 1
                  for j in range(n_blocks):
                      ncol = slice(j * nb, (j + 1) * nb)
                      it = gi * n_blocks + j
                      if _rep == 0 and it in q_weave:
                          qk_piece_pe("q", q_dram, qt_all, *q_weave[it])
                      pool = sgpa if it % 2 == 0 else sgpb
                      s_g = pool.tile(
                          [P, g, nb], F32, tag=f"sg{it % 2}", name=f"sg{it % 2}"
                      )
                      for ci in range(g):
                          cc = mc + ci
                          nc.tensor.matmul(
                              s_g[:, ci, :],
                              kt_all[0:D, cc * P : (cc + 1) * P],
                              qt_all[0:D, ncol],
                              start=True,
                              stop=True,
                          )
                      p_g = pexp.tile([P, g, nb], BF16, tag="pg")
                      nc.scalar.activation(
                          out=p_g,
                          in_=s_g,
                          func=mybir.ActivationFunctionType.Exp,
                          scale=float(scale),
                      )
                      part = partp.tile([P, ntiles_blk, P], F32, tag="part")
                      for ci in range(g):
                          cc = mc + ci
                          for t in range(ntiles_blk):
                              nc.tensor.matmul(
                                  part[:, t, 0 : D + 1],
                                  p_g[:, ci, t * P : (t + 1) * P],
                                  v2[:, cc, :],
                                  start=(ci == 0 and t == 0),
                                  stop=(ci == g - 1 and t == ntiles_blk - 1),
                                  skip_group_check=True,
                              )
                      if gi == 0:
                          nc.vector.tensor_copy(oaccs[j], part[:, :, 0 : D + 1])
                      else:
                          nc.vector.tensor_add(
                              oaccs[j], oaccs[j], part[:, :, 0 : D + 1]
                          )

                      if last_group:
                          # epilogue for block j, streams with the inner loop
                          rinv = small.tile([P, ntiles_blk, 1], F32, tag="rinv")
                          nc.vector.reciprocal(rinv, oaccs[j][:, :, D : D + 1])
                          o_sb = outp.tile([P, ntiles_blk, D], F32, tag="osb")
                          rinv_b = bass.AP(
                              tensor=rinv.tensor,
                              offset=rinv.offset,
                              ap=[rinv.ap[0], rinv.ap[1], [0, D]],
                          )
                          nc.vector.tensor_tensor(
                              out=o_sb,
                              in0=oaccs[j][:, :, 0:D],
                              in1=rinv_b,
                              op=mybir.AluOpType.mult,
                          )
                          nc.sync.dma_start(
                              out=o_tiled[:, j * ntiles_blk : (j + 1) * ntiles_blk, :],
                              in_=o_sb,
                          )
                  mc += g

    nc.compile()
    return nc


_NC_CACHE = {}


def _get_nc(**kw):
    key = tuple(sorted(kw.items()))
    if key not in _NC_CACHE:
        _NC_CACHE[key] = build_nc(**kw)
    return _NC_CACHE[key]


def kernel(queries, keys, values, **run_kwargs):
    """Full-input entry point: [8, 4096, 64] fp32 each -> [8, 4096, 64] fp32."""
    nc = _get_nc()
    in_maps = [
        {
            "queries": np.ascontiguousarray(queries[b], dtype=np.float32),
            "keys": np.ascontiguousarray(keys[b], dtype=np.float32),
            "values": np.ascontiguousarray(values[b], dtype=np.float32),
        }
        for b in range(B)
    ]
    res = run_bass_kernel_spmd(nc, in_maps, core_ids=list(range(B)), **run_kwargs)
    out = np.stack([res.results[b]["out"] for b in range(B)]).astype(np.float32)
    if run_kwargs:
        kernel.last_results = res
    return out

